# revision 7
# baseline (speedup 1.0000x reference)
"""Multi-head attention (B=2, N=2048, C=1024, H=16, D=64) on 8 Trainium2
NeuronCores.

Sharding: tensor-parallel over heads x data-parallel over batch.
Core (b, g) with b in {0,1}, g in {0..3} handles batch b and heads
[4g, 4g+4). Each core computes qkv for its heads, attention, and a partial
output projection (row-parallel); the host sums the 4 partials per batch and
adds the bias.

Per-core kernel layout (all matmuls in fp32r — full fp32 storage, reduced
multiplier precision, 1 PE cycle/row):
  qT/kT [d, n] via lhsT=w^T, rhs=x^T          (d on partitions, pair-packed)
  scoresT[j, i] = kT.T @ qT                   (two K=64 row-tiled matmuls)
  attnT = exp(scale * scoresT)                (ACT, PSUM->SBUF, no max pass)
  aoT[d, i] += [v | 1]^T @ attnT              (row 64 = softmax denominators)
  aoT *= 1/sums (broadcast), then out = aoT.T @ wpT partial projection.
"""
import numpy as np
import sys

sys.path.insert(0, "/opt/trn_rl_repo")

B = 2
N = 2048
C = 1024
H = 16
D = 64
SCALE = D ** -0.5

HEADS_PER_CORE = 4  # 2 pairs
N_CORES = 8

_cache = {}


def _build():
    import concourse.bass as bass
    import concourse.tile as tile
    from concourse import bacc, mybir

    F32 = mybir.dt.float32
    F32R = mybir.dt.float32r
    P = 128
    NC4 = N // 512   # 4 i-chunks of 512
    NB = N // P      # 16 n/j blocks of 128
    CO = C // P      # 8 contraction subtiles

    nc = bacc.Bacc("TRN2", target_bir_lowering=False, debug=False)
    xT = nc.dram_tensor("xT", (C, N), F32, kind="ExternalInput")
    wqkT = nc.dram_tensor("wqkT", (C, 512), F32, kind="ExternalInput")
    wvT = nc.dram_tensor("wvT", (C, 256), F32, kind="ExternalInput")
    wpT = nc.dram_tensor("wpT", (256, C), F32, kind="ExternalInput")
    out = nc.dram_tensor("out", (N, C), F32, kind="ExternalOutput")

    with tile.TileContext(nc) as tc:
        with (
            tc.tile_pool(name="big", bufs=1) as big,
            tc.tile_pool(name="attn", bufs=3) as attn_pool,
            tc.tile_pool(name="norm", bufs=2) as norm_pool,
            tc.tile_pool(name="outp", bufs=3) as out_pool,
            tc.tile_pool(name="ps_mm", bufs=2, space="PSUM") as ps_mm,
            tc.tile_pool(name="ps_sc", bufs=2, space="PSUM") as ps_sc,
            tc.tile_pool(name="ps_av", bufs=1, space="PSUM") as ps_av,
        ):
            # ---- weights + x loads ----
            wqk_sb = big.tile([P, CO, 512], F32R)
            for co in range(CO):
                nc.sync.dma_start(
                    wqk_sb[:, co, :],
                    wqkT.ap()[co * P:(co + 1) * P, :].bitcast(F32R),
                )
            wv_sb = big.tile([P, CO, 256], F32R)
            for co in range(CO):
                nc.sync.dma_start(
                    wv_sb[:, co, :],
                    wvT.ap()[co * P:(co + 1) * P, :].bitcast(F32R),
                )
            wp_sb = big.tile([P, 2, C], F32R)
            for cs in range(2):
                nc.sync.dma_start(
                    wp_sb[:, cs, :],
                    wpT.ap()[cs * P:(cs + 1) * P, :].bitcast(F32R),
                )
            ones_c = big.tile([P, 1], F32)
            nc.vector.memset(ones_c[:], 1.0)

            xT_sb = big.tile([P, CO, N], F32R)
            for co in range(CO):
                nc.sync.dma_start(
                    xT_sb[:, co, :],
                    xT.ap()[co * P:(co + 1) * P, :].bitcast(F32R),
                )

            # ---- qT / kT (pair-packed [d(2x64), n]) ----
            # wqkT cols: [q_p0 | k_p0 | q_p1 | k_p1] each 128 wide
            qk_sb = [big.tile([P, N], F32R, name=f"qk_sb{i}") for i in range(4)]
            for fc in range(4):
                for ick in range(NC4):
                    pm = ps_mm.tile([P, 512], F32)
                    for co in range(CO):
                        nc.tensor.matmul(
                            pm[:],
                            wqk_sb[:, co, fc * P:(fc + 1) * P],
                            xT_sb[:, co, ick * 512:(ick + 1) * 512],
                            start=(co == 0),
                            stop=(co == CO - 1),
                        )
                    nc.vector.tensor_copy(
                        qk_sb[fc][:, ick * 512:(ick + 1) * 512], pm[:]
                    )

            # ---- v in natural layout [n(j), d] + ones column ----
            v_ones = big.tile([P, NB, HEADS_PER_CORE, 65], F32R)
            nc.vector.tensor_copy(
                v_ones[:, :, :, 64:65],
                ones_c.unsqueeze(1).unsqueeze(1).to_broadcast(
                    (P, NB, HEADS_PER_CORE, 1)
                ),
            )
            for nb in range(NB):
                pm = ps_mm.tile([P, 512], F32)
                for co in range(CO):
                    nc.tensor.matmul(
                        pm[:, 0:256],
                        xT_sb[:, co, nb * P:(nb + 1) * P],
                        wv_sb[:, co, :],
                        start=(co == 0),
                        stop=(co == CO - 1),
                    )
                nc.vector.tensor_copy(
                    v_ones[:, nb, :, 0:64],
                    pm[:, 0:256].rearrange("p (h d) -> p h d", h=HEADS_PER_CORE),
                )

            # ---- attention per pair, per i-chunk ----
            # qk_sb index: q of pair p -> 2*p, k of pair p -> 2*p+1
            aoT_sb = [big.tile([P, N], F32R, name=f"aoT_sb{i}") for i in range(2)]
            for pair in range(2):
                q_t = qk_sb[2 * pair]
                k_t = qk_sb[2 * pair + 1]
                hA = 2 * pair
                hB = 2 * pair + 1
                for ick in range(NC4):
                    isl = slice(ick * 512, (ick + 1) * 512)
                    av_A = ps_av.tile([65, 512], F32)
                    av_B = ps_av.tile([65, 512], F32)
                    for jb in range(NB):
                        jsl = slice(jb * P, (jb + 1) * P)
                        sc = ps_sc.tile([P, 2, 512], F32)
                        nc.tensor.matmul(
                            sc[:, 0, :], k_t[0:64, jsl], q_t[0:64, isl],
                            start=True, stop=True,
                        )
                        nc.tensor.matmul(
                            sc[:, 1, :], k_t[64:128, jsl], q_t[64:128, isl],
                            start=True, stop=True,
                        )
                        at = attn_pool.tile([P, 2, 512], F32R)
                        nc.scalar.activation(
                            out=at[:], in_=sc[:],
                            func=mybir.ActivationFunctionType.Exp,
                            scale=float(SCALE),
                        )
                        nc.tensor.matmul(
                            av_A[:], v_ones[:, jb, hA, :], at[:, 0, :],
                            start=(jb == 0), stop=(jb == NB - 1),
                        )
                        nc.tensor.matmul(
                            av_B[:], v_ones[:, jb, hB, :], at[:, 1, :],
                            start=(jb == 0), stop=(jb == NB - 1),
                        )
                    # normalize: aoT[d, i] /= sums[i] (row 64 of av psum)
                    sumsA = norm_pool.tile([1, 512], F32)
                    sumsB = norm_pool.tile([1, 512], F32)
                    nc.vector.tensor_copy(sumsA[:], av_A[64:65, :])
                    nc.vector.tensor_copy(sumsB[:], av_B[64:65, :])
                    recA = norm_pool.tile([1, 512], F32)
                    recB = norm_pool.tile([1, 512], F32)
                    nc.vector.reciprocal_approx_fast(out=recA[:], in_=sumsA[:])
                    nc.vector.reciprocal_approx_fast(out=recB[:], in_=sumsB[:])
                    rbcA = norm_pool.tile([64, 512], F32)
                    rbcB = norm_pool.tile([64, 512], F32)
                    nc.gpsimd.partition_broadcast(rbcA[:], recA[:])
                    nc.gpsimd.partition_broadcast(rbcB[:], recB[:])
                    nc.vector.tensor_mul(
                        aoT_sb[pair][0:64, isl], av_A[0:64, :], rbcA[:]
                    )
                    nc.vector.tensor_mul(
                        aoT_sb[pair][64:128, isl], av_B[0:64, :], rbcB[:]
                    )

            # ---- output projection (partial over this core's 256 channels) ----
            for nb in range(NB):
                nsl = slice(nb * P, (nb + 1) * P)
                for fck in range(2):
                    fsl = slice(fck * 512, (fck + 1) * 512)
                    pm = ps_mm.tile([P, 512], F32)
                    nc.tensor.matmul(
                        pm[:], aoT_sb[0][:, nsl], wp_sb[:, 0, fsl],
                        start=True, stop=False,
                    )
                    nc.tensor.matmul(
                        pm[:], aoT_sb[1][:, nsl], wp_sb[:, 1, fsl],
                        start=False, stop=True,
                    )
                    ot = out_pool.tile([P, 512], F32)
                    nc.vector.tensor_copy(ot[:], pm[:])
                    nc.sync.dma_start(out.ap()[nsl, fsl], ot[:])

    nc.compile()
    return nc


def _get_nc():
    if "nc" not in _cache:
        _cache["nc"] = _build()
    return _cache["nc"]


def _shard_inputs(x, w_qkv, w_proj):
    """Build per-core input dicts. Core index = b * 4 + g."""
    in_maps = []
    for b in range(B):
        xTb = np.ascontiguousarray(x[b].T)  # [C, N]
        for g in range(4):
            r = g * 256  # head-group row offset within each of q/k/v sections
            wqkT = np.empty((C, 512), np.float32)
            wqkT[:, 0:128] = w_qkv[r:r + 128].T                  # q pair 0
            wqkT[:, 128:256] = w_qkv[C + r:C + r + 128].T        # k pair 0
            wqkT[:, 256:384] = w_qkv[r + 128:r + 256].T          # q pair 1
            wqkT[:, 384:512] = w_qkv[C + r + 128:C + r + 256].T  # k pair 1
            wvT = np.ascontiguousarray(w_qkv[2 * C + r:2 * C + r + 256].T)
            wpT = np.ascontiguousarray(w_proj[:, r:r + 256].T)
            in_maps.append({
                "xT": xTb,
                "wqkT": wqkT,
                "wvT": wvT,
                "wpT": wpT,
            })
    return in_maps


def kernel(x, w_qkv, w_proj, b_proj, _trace=False):
    from concourse.bass_utils import run_bass_kernel_spmd

    x = np.asarray(x, dtype=np.float32)
    w_qkv = np.asarray(w_qkv, dtype=np.float32)
    w_proj = np.asarray(w_proj, dtype=np.float32)
    b_proj = np.asarray(b_proj, dtype=np.float32)

    nc = _get_nc()
    in_maps = _shard_inputs(x, w_qkv, w_proj)
    res = run_bass_kernel_spmd(
        nc, in_maps, core_ids=list(range(N_CORES)), trace=_trace
    )
    out = np.zeros((B, N, C), np.float32)
    for b in range(B):
        for g in range(4):
            out[b] += res.results[b * 4 + g]["out"]
    out += b_proj
    if _trace:
        _cache["last_exec_time_ns"] = res.exec_time_ns
        _cache["last_results"] = res
    return out


# revision 17
# speedup vs baseline: 1.0720x; 1.0720x over previous
"""Multi-head attention (B=2, N=2048, C=1024, H=16, D=64) on 8 Trainium2
NeuronCores.

Sharding: tensor-parallel over heads x data-parallel over batch.
Core (b, g) with b in {0,1}, g in {0..3} handles batch b and heads
[4g, 4g+4). Each core computes qkv for its heads, attention, and a partial
output projection (row-parallel); the host sums the 4 partials per batch and
adds the bias.

Per-core kernel layout (all matmuls in fp32r — full fp32 storage, reduced
multiplier precision, 1 PE cycle/row):
  qT/kT [d, n] via lhsT=w^T, rhs=x^T          (d on partitions, pair-packed)
  scoresT[j, i] = kT.T @ qT                   (two K=64 row-tiled matmuls)
  attnT = exp(scale * scoresT)                (ACT, PSUM->SBUF, no max pass)
  aoT[d, i] += [v | 1]^T @ attnT              (row 64 = softmax denominators)
  aoT *= 1/sums (broadcast), then out = aoT.T @ wpT partial projection.
"""
import numpy as np
import sys

sys.path.insert(0, "/opt/trn_rl_repo")

B = 2
N = 2048
C = 1024
H = 16
D = 64
SCALE = D ** -0.5

HEADS_PER_CORE = 4  # 2 pairs
N_CORES = 8

_cache = {}


def _build():
    import concourse.bass as bass
    import concourse.tile as tile
    from concourse import bacc, mybir

    F32 = mybir.dt.float32
    F32R = mybir.dt.float32r
    P = 128
    NC4 = N // 512   # 4 i-chunks of 512
    NB = N // P      # 16 n/j blocks of 128
    CO = C // P      # 8 contraction subtiles

    nc = bacc.Bacc("TRN2", target_bir_lowering=False, debug=False)
    xT = nc.dram_tensor("xT", (C, N), F32, kind="ExternalInput")
    wqkT = nc.dram_tensor("wqkT", (C, 512), F32, kind="ExternalInput")
    wvT = nc.dram_tensor("wvT", (C, 256), F32, kind="ExternalInput")
    wpT = nc.dram_tensor("wpT", (256, C), F32, kind="ExternalInput")
    out = nc.dram_tensor("out", (N, C), F32, kind="ExternalOutput")

    with tile.TileContext(nc) as tc:
        with (
            tc.tile_pool(name="big", bufs=1) as big,
            tc.tile_pool(name="attn", bufs=3) as attn_pool,
            tc.tile_pool(name="norm", bufs=2) as norm_pool,
            tc.tile_pool(name="outp", bufs=3) as out_pool,
            tc.tile_pool(name="ps_mm", bufs=2, space="PSUM") as ps_mm,
            tc.tile_pool(name="ps_sc", bufs=2, space="PSUM") as ps_sc,
            tc.tile_pool(name="ps_av", bufs=1, space="PSUM") as ps_av,
        ):
            # ---- weights + x loads ----
            wqk_sb = big.tile([P, CO, 512], F32R)
            for co in range(CO):
                nc.sync.dma_start(
                    wqk_sb[:, co, :],
                    wqkT.ap()[co * P:(co + 1) * P, :].bitcast(F32R),
                )
            wv_sb = big.tile([P, CO, 256], F32R)
            for co in range(CO):
                nc.sync.dma_start(
                    wv_sb[:, co, :],
                    wvT.ap()[co * P:(co + 1) * P, :].bitcast(F32R),
                )
            wp_sb = big.tile([P, 2, C], F32R)
            for cs in range(2):
                nc.sync.dma_start(
                    wp_sb[:, cs, :],
                    wpT.ap()[cs * P:(cs + 1) * P, :].bitcast(F32R),
                )
            ones_c = big.tile([P, 1], F32)
            nc.vector.memset(ones_c[:], 1.0)

            xT_sb = big.tile([P, CO, N], F32R)
            for co in range(CO):
                nc.sync.dma_start(
                    xT_sb[:, co, :],
                    xT.ap()[co * P:(co + 1) * P, :].bitcast(F32R),
                )

            # PE warm-up: ~8us of junk matmuls on a zeroed tile while input
            # DMAs stream, so the HAM clock-gate is at 8/8 when real work
            # starts. Results go to a scratch psum that is never read.
            import os as _os
            _warmup = _os.environ.get("K_NO_WARMUP") != "1"
            warm = big.tile([P, 512], F32R)
            nc.vector.memset(warm[:].bitcast(F32), 0.0)
            wsink = big.tile([P, 512], F32)
            for wu in range(36 if _warmup else 0):
                pw = ps_mm.tile([P, 512], F32, name="pwarm", tag="pm")
                nc.tensor.matmul(
                    pw[:], warm[:, 0:128], warm[:], start=True, stop=True
                )
                if wu % 18 == 17:
                    nc.vector.tensor_copy(wsink[:], pw[:])

            # ---- qT / kT (pair-packed [d(2x64), n]) ----
            # wqkT cols: [q_p0 | k_p0 | q_p1 | k_p1] each 128 wide
            qk_sb = [big.tile([P, N], F32R, name=f"qk_sb{i}") for i in range(4)]
            for fc in range(4):
                for ick in range(NC4):
                    pm = ps_mm.tile([P, 512], F32)
                    for co in range(CO):
                        nc.tensor.matmul(
                            pm[:],
                            wqk_sb[:, co, fc * P:(fc + 1) * P],
                            xT_sb[:, co, ick * 512:(ick + 1) * 512],
                            start=(co == 0),
                            stop=(co == CO - 1),
                        )
                    nc.vector.tensor_copy(
                        qk_sb[fc][:, ick * 512:(ick + 1) * 512], pm[:]
                    )

            # ---- v in natural layout [n(j), d] + ones column ----
            v_ones = big.tile([P, NB, HEADS_PER_CORE, 65], F32R)
            nc.vector.tensor_copy(
                v_ones[:, :, :, 64:65],
                ones_c.unsqueeze(1).unsqueeze(1).to_broadcast(
                    (P, NB, HEADS_PER_CORE, 1)
                ),
            )
            for nb in range(NB):
                pm = ps_mm.tile([P, 512], F32)
                for co in range(CO):
                    nc.tensor.matmul(
                        pm[:, 0:256],
                        xT_sb[:, co, nb * P:(nb + 1) * P],
                        wv_sb[:, co, :],
                        start=(co == 0),
                        stop=(co == CO - 1),
                    )
                nc.vector.tensor_copy(
                    v_ones[:, nb, :, 0:64],
                    pm[:, 0:256].rearrange("p (h d) -> p h d", h=HEADS_PER_CORE),
                )

            # ---- attention per pair, per i-chunk ----
            # qk_sb index: q of pair p -> 2*p, k of pair p -> 2*p+1
            aoT_sb = [big.tile([P, N], F32R, name=f"aoT_sb{i}") for i in range(2)]
            for pair in range(2):
                q_t = qk_sb[2 * pair]
                k_t = qk_sb[2 * pair + 1]
                hA = 2 * pair
                hB = 2 * pair + 1
                for ick in range(NC4):
                    isl = slice(ick * 512, (ick + 1) * 512)
                    av_A = ps_av.tile([65, 512], F32)
                    av_B = ps_av.tile([65, 512], F32)
                    for jb in range(NB):
                        jsl = slice(jb * P, (jb + 1) * P)
                        sc = ps_sc.tile([P, 2, 512], F32)
                        nc.tensor.matmul(
                            sc[:, 0, :], k_t[0:64, jsl], q_t[0:64, isl],
                            start=True, stop=True,
                        )
                        nc.tensor.matmul(
                            sc[:, 1, :], k_t[64:128, jsl], q_t[64:128, isl],
                            start=True, stop=True,
                        )
                        at = attn_pool.tile([P, 2, 512], F32R)
                        nc.scalar.activation(
                            out=at[:], in_=sc[:],
                            func=mybir.ActivationFunctionType.Exp,
                            scale=float(SCALE),
                        )
                        nc.tensor.matmul(
                            av_A[:], v_ones[:, jb, hA, :], at[:, 0, :],
                            start=(jb == 0), stop=(jb == NB - 1),
                        )
                        nc.tensor.matmul(
                            av_B[:], v_ones[:, jb, hB, :], at[:, 1, :],
                            start=(jb == 0), stop=(jb == NB - 1),
                        )
                    # Copy av psums to SBUF right away so the PSUM banks free
                    # for the next i-chunk; normalize from SBUF off the
                    # critical path: aoT[d, i] /= sums[i] (row 64 = sums).
                    # Release the av psum banks promptly: copy unnormalized
                    # aoT + sums to SBUF, then normalize aoT in place.
                    sumsA = norm_pool.tile([1, 512], F32)
                    sumsB = norm_pool.tile([1, 512], F32)
                    nc.vector.tensor_copy(aoT_sb[pair][0:64, isl], av_A[0:64, :])
                    nc.vector.tensor_copy(aoT_sb[pair][64:128, isl], av_B[0:64, :])
                    nc.vector.tensor_copy(sumsA[:], av_A[64:65, :])
                    nc.vector.tensor_copy(sumsB[:], av_B[64:65, :])
                    recA = norm_pool.tile([1, 512], F32)
                    recB = norm_pool.tile([1, 512], F32)
                    nc.vector.reciprocal_approx_fast(out=recA[:], in_=sumsA[:])
                    nc.vector.reciprocal_approx_fast(out=recB[:], in_=sumsB[:])
                    rbcA = norm_pool.tile([64, 512], F32)
                    rbcBhi = norm_pool.tile([P, 512], F32)
                    nc.gpsimd.partition_broadcast(rbcA[:], recA[:])
                    nc.gpsimd.partition_broadcast(rbcBhi[0:64, :], recB[:])
                    # DVE SBUF+SBUF inputs must share base partition; shift
                    # head B's recip rows up to partitions 64-127 first.
                    nc.vector.tensor_copy(rbcBhi[64:128, :], rbcBhi[0:64, :])
                    nc.vector.tensor_mul(
                        aoT_sb[pair][0:64, isl], aoT_sb[pair][0:64, isl], rbcA[:]
                    )
                    nc.vector.tensor_mul(
                        aoT_sb[pair][64:128, isl],
                        aoT_sb[pair][64:128, isl],
                        rbcBhi[64:128, :],
                    )

            # ---- output projection (partial over this core's 256 channels) ----
            for nb in range(NB):
                nsl = slice(nb * P, (nb + 1) * P)
                for fck in range(2):
                    fsl = slice(fck * 512, (fck + 1) * 512)
                    pm = ps_mm.tile([P, 512], F32)
                    nc.tensor.matmul(
                        pm[:], aoT_sb[0][:, nsl], wp_sb[:, 0, fsl],
                        start=True, stop=False,
                    )
                    nc.tensor.matmul(
                        pm[:], aoT_sb[1][:, nsl], wp_sb[:, 1, fsl],
                        start=False, stop=True,
                    )
                    ot = out_pool.tile([P, 512], F32)
                    nc.vector.tensor_copy(ot[:], pm[:])
                    nc.sync.dma_start(out.ap()[nsl, fsl], ot[:])

    nc.compile()
    return nc


def _get_nc():
    if "nc" not in _cache:
        _cache["nc"] = _build()
    return _cache["nc"]


def _shard_inputs(x, w_qkv, w_proj):
    """Build per-core input dicts. Core index = b * 4 + g."""
    in_maps = []
    for b in range(B):
        xTb = np.ascontiguousarray(x[b].T)  # [C, N]
        for g in range(4):
            r = g * 256  # head-group row offset within each of q/k/v sections
            wqkT = np.empty((C, 512), np.float32)
            wqkT[:, 0:128] = w_qkv[r:r + 128].T                  # q pair 0
            wqkT[:, 128:256] = w_qkv[C + r:C + r + 128].T        # k pair 0
            wqkT[:, 256:384] = w_qkv[r + 128:r + 256].T          # q pair 1
            wqkT[:, 384:512] = w_qkv[C + r + 128:C + r + 256].T  # k pair 1
            wvT = np.ascontiguousarray(w_qkv[2 * C + r:2 * C + r + 256].T)
            wpT = np.ascontiguousarray(w_proj[:, r:r + 256].T)
            in_maps.append({
                "xT": xTb,
                "wqkT": wqkT,
                "wvT": wvT,
                "wpT": wpT,
            })
    return in_maps


def kernel(x, w_qkv, w_proj, b_proj, _trace=False):
    from concourse.bass_utils import run_bass_kernel_spmd

    x = np.asarray(x, dtype=np.float32)
    w_qkv = np.asarray(w_qkv, dtype=np.float32)
    w_proj = np.asarray(w_proj, dtype=np.float32)
    b_proj = np.asarray(b_proj, dtype=np.float32)

    nc = _get_nc()
    in_maps = _shard_inputs(x, w_qkv, w_proj)
    res = run_bass_kernel_spmd(
        nc, in_maps, core_ids=list(range(N_CORES)), trace=_trace
    )
    out = np.zeros((B, N, C), np.float32)
    for b in range(B):
        for g in range(4):
            out[b] += res.results[b * 4 + g]["out"]
    out += b_proj
    if _trace:
        _cache["last_exec_time_ns"] = res.exec_time_ns
        _cache["last_results"] = res
    return out


# revision 18
# speedup vs baseline: 1.1021x; 1.0281x over previous
"""Multi-head attention (B=2, N=2048, C=1024, H=16, D=64) on 8 Trainium2
NeuronCores.

Sharding: tensor-parallel over heads x data-parallel over batch.
Core (b, g) with b in {0,1}, g in {0..3} handles batch b and heads
[4g, 4g+4). Each core computes qkv for its heads, attention, and a partial
output projection (row-parallel); the host sums the 4 partials per batch and
adds the bias.

Per-core kernel layout (all matmuls in fp32r — full fp32 storage, reduced
multiplier precision, 1 PE cycle/row):
  qT/kT [d, n] via lhsT=w^T, rhs=x^T          (d on partitions, pair-packed)
  scoresT[j, i] = kT.T @ qT                   (two K=64 row-tiled matmuls)
  attnT = exp(scale * scoresT)                (ACT, PSUM->SBUF, no max pass)
  aoT[d, i] += [v | 1]^T @ attnT              (row 64 = softmax denominators)
  aoT *= 1/sums (broadcast), then out = aoT.T @ wpT partial projection.
"""
import numpy as np
import sys

sys.path.insert(0, "/opt/trn_rl_repo")

B = 2
N = 2048
C = 1024
H = 16
D = 64
SCALE = D ** -0.5

HEADS_PER_CORE = 4  # 2 pairs
N_CORES = 8

_cache = {}


def _build():
    import concourse.bass as bass
    import concourse.tile as tile
    from concourse import bacc, mybir

    F32 = mybir.dt.float32
    F32R = mybir.dt.float32r
    P = 128
    NC4 = N // 512   # 4 i-chunks of 512
    NB = N // P      # 16 n/j blocks of 128
    CO = C // P      # 8 contraction subtiles

    nc = bacc.Bacc("TRN2", target_bir_lowering=False, debug=False)
    xT = nc.dram_tensor("xT", (C, N), F32, kind="ExternalInput")
    wqkT = nc.dram_tensor("wqkT", (C, 512), F32, kind="ExternalInput")
    wvT = nc.dram_tensor("wvT", (C, 256), F32, kind="ExternalInput")
    wpT = nc.dram_tensor("wpT", (256, C), F32, kind="ExternalInput")
    out = nc.dram_tensor("out", (N, C), F32, kind="ExternalOutput")

    with tile.TileContext(nc) as tc:
        with (
            tc.tile_pool(name="big", bufs=1) as big,
            tc.tile_pool(name="attn", bufs=3) as attn_pool,
            tc.tile_pool(name="norm", bufs=2) as norm_pool,
            tc.tile_pool(name="outp", bufs=3) as out_pool,
            tc.tile_pool(name="ps_mm", bufs=2, space="PSUM") as ps_mm,
            tc.tile_pool(name="ps_sc", bufs=2, space="PSUM") as ps_sc,
            tc.tile_pool(name="ps_av", bufs=1, space="PSUM") as ps_av,
        ):
            # ---- weights + x loads ----
            wqk_sb = big.tile([P, CO, 512], F32R)
            for co in range(CO):
                nc.sync.dma_start(
                    wqk_sb[:, co, :],
                    wqkT.ap()[co * P:(co + 1) * P, :].bitcast(F32R),
                )
            wv_sb = big.tile([P, CO, 256], F32R)
            for co in range(CO):
                nc.sync.dma_start(
                    wv_sb[:, co, :],
                    wvT.ap()[co * P:(co + 1) * P, :].bitcast(F32R),
                )
            wp_sb = big.tile([P, 2, C], F32R)
            for cs in range(2):
                nc.sync.dma_start(
                    wp_sb[:, cs, :],
                    wpT.ap()[cs * P:(cs + 1) * P, :].bitcast(F32R),
                )
            ones_c = big.tile([P, 1], F32)
            nc.vector.memset(ones_c[:], 1.0)
            # Preload the exp ACT table now (~2.7us) so the first real exp in
            # the attention phase doesn't stall the PE past the HAM window.
            exp_warm = big.tile([P, 1], F32)
            nc.scalar.activation(
                out=exp_warm[:], in_=ones_c[:],
                func=mybir.ActivationFunctionType.Exp,
            )

            xT_sb = big.tile([P, CO, N], F32R)
            for co in range(CO):
                nc.sync.dma_start(
                    xT_sb[:, co, :],
                    xT.ap()[co * P:(co + 1) * P, :].bitcast(F32R),
                )

            # PE warm-up: ~8us of junk matmuls on a zeroed tile while input
            # DMAs stream, so the HAM clock-gate is at 8/8 when real work
            # starts. Results go to a scratch psum that is never read.
            import os as _os
            _warmup = _os.environ.get("K_NO_WARMUP") != "1"
            warm = big.tile([P, 512], F32R)
            nc.vector.memset(warm[:].bitcast(F32), 0.0)
            wsink = big.tile([P, 512], F32)
            for wu in range(36 if _warmup else 0):
                pw = ps_mm.tile([P, 512], F32, name="pwarm", tag="pm")
                nc.tensor.matmul(
                    pw[:], warm[:, 0:128], warm[:], start=True, stop=True
                )
                if wu % 18 == 17:
                    nc.vector.tensor_copy(wsink[:], pw[:])

            # ---- qT / kT (pair-packed [d(2x64), n]) ----
            # wqkT cols: [q_p0 | k_p0 | q_p1 | k_p1] each 128 wide
            qk_sb = [big.tile([P, N], F32R, name=f"qk_sb{i}") for i in range(4)]
            for fc in range(4):
                for ick in range(NC4):
                    pm = ps_mm.tile([P, 512], F32)
                    for co in range(CO):
                        nc.tensor.matmul(
                            pm[:],
                            wqk_sb[:, co, fc * P:(fc + 1) * P],
                            xT_sb[:, co, ick * 512:(ick + 1) * 512],
                            start=(co == 0),
                            stop=(co == CO - 1),
                        )
                    nc.vector.tensor_copy(
                        qk_sb[fc][:, ick * 512:(ick + 1) * 512], pm[:]
                    )

            # ---- v in natural layout [n(j), d] + ones column ----
            v_ones = big.tile([P, NB, HEADS_PER_CORE, 65], F32R)
            nc.vector.tensor_copy(
                v_ones[:, :, :, 64:65],
                ones_c.unsqueeze(1).unsqueeze(1).to_broadcast(
                    (P, NB, HEADS_PER_CORE, 1)
                ),
            )
            for nb in range(NB):
                pm = ps_mm.tile([P, 512], F32)
                for co in range(CO):
                    nc.tensor.matmul(
                        pm[:, 0:256],
                        xT_sb[:, co, nb * P:(nb + 1) * P],
                        wv_sb[:, co, :],
                        start=(co == 0),
                        stop=(co == CO - 1),
                    )
                nc.vector.tensor_copy(
                    v_ones[:, nb, :, 0:64],
                    pm[:, 0:256].rearrange("p (h d) -> p h d", h=HEADS_PER_CORE),
                )

            # ---- attention per pair, per i-chunk ----
            # qk_sb index: q of pair p -> 2*p, k of pair p -> 2*p+1
            aoT_sb = [big.tile([P, N], F32R, name=f"aoT_sb{i}") for i in range(2)]
            for pair in range(2):
                q_t = qk_sb[2 * pair]
                k_t = qk_sb[2 * pair + 1]
                hA = 2 * pair
                hB = 2 * pair + 1
                for ick in range(NC4):
                    isl = slice(ick * 512, (ick + 1) * 512)
                    av_A = ps_av.tile([65, 512], F32)
                    av_B = ps_av.tile([65, 512], F32)
                    for jb in range(NB):
                        jsl = slice(jb * P, (jb + 1) * P)
                        sc = ps_sc.tile([P, 2, 512], F32)
                        nc.tensor.matmul(
                            sc[:, 0, :], k_t[0:64, jsl], q_t[0:64, isl],
                            start=True, stop=True,
                        )
                        nc.tensor.matmul(
                            sc[:, 1, :], k_t[64:128, jsl], q_t[64:128, isl],
                            start=True, stop=True,
                        )
                        at = attn_pool.tile([P, 2, 512], F32R)
                        nc.scalar.activation(
                            out=at[:], in_=sc[:],
                            func=mybir.ActivationFunctionType.Exp,
                            scale=float(SCALE),
                        )
                        nc.tensor.matmul(
                            av_A[:], v_ones[:, jb, hA, :], at[:, 0, :],
                            start=(jb == 0), stop=(jb == NB - 1),
                        )
                        nc.tensor.matmul(
                            av_B[:], v_ones[:, jb, hB, :], at[:, 1, :],
                            start=(jb == 0), stop=(jb == NB - 1),
                        )
                    # Copy av psums to SBUF right away so the PSUM banks free
                    # for the next i-chunk; normalize from SBUF off the
                    # critical path: aoT[d, i] /= sums[i] (row 64 = sums).
                    # Release the av psum banks promptly: copy unnormalized
                    # aoT + sums to SBUF, then normalize aoT in place.
                    sumsA = norm_pool.tile([1, 512], F32)
                    sumsB = norm_pool.tile([1, 512], F32)
                    nc.vector.tensor_copy(aoT_sb[pair][0:64, isl], av_A[0:64, :])
                    nc.vector.tensor_copy(aoT_sb[pair][64:128, isl], av_B[0:64, :])
                    nc.vector.tensor_copy(sumsA[:], av_A[64:65, :])
                    nc.vector.tensor_copy(sumsB[:], av_B[64:65, :])
                    recA = norm_pool.tile([1, 512], F32)
                    recB = norm_pool.tile([1, 512], F32)
                    nc.vector.reciprocal_approx_fast(out=recA[:], in_=sumsA[:])
                    nc.vector.reciprocal_approx_fast(out=recB[:], in_=sumsB[:])
                    rbcA = norm_pool.tile([64, 512], F32)
                    rbcBhi = norm_pool.tile([P, 512], F32)
                    nc.gpsimd.partition_broadcast(rbcA[:], recA[:])
                    nc.gpsimd.partition_broadcast(rbcBhi[0:64, :], recB[:])
                    # DVE SBUF+SBUF inputs must share base partition; shift
                    # head B's recip rows up to partitions 64-127 first.
                    nc.vector.tensor_copy(rbcBhi[64:128, :], rbcBhi[0:64, :])
                    nc.vector.tensor_mul(
                        aoT_sb[pair][0:64, isl], aoT_sb[pair][0:64, isl], rbcA[:]
                    )
                    nc.vector.tensor_mul(
                        aoT_sb[pair][64:128, isl],
                        aoT_sb[pair][64:128, isl],
                        rbcBhi[64:128, :],
                    )

            # ---- output projection (partial over this core's 256 channels) ----
            for nb in range(NB):
                nsl = slice(nb * P, (nb + 1) * P)
                for fck in range(2):
                    fsl = slice(fck * 512, (fck + 1) * 512)
                    pm = ps_mm.tile([P, 512], F32)
                    nc.tensor.matmul(
                        pm[:], aoT_sb[0][:, nsl], wp_sb[:, 0, fsl],
                        start=True, stop=False,
                    )
                    nc.tensor.matmul(
                        pm[:], aoT_sb[1][:, nsl], wp_sb[:, 1, fsl],
                        start=False, stop=True,
                    )
                    ot = out_pool.tile([P, 512], F32)
                    nc.vector.tensor_copy(ot[:], pm[:])
                    nc.sync.dma_start(out.ap()[nsl, fsl], ot[:])

    nc.compile()
    return nc


def _get_nc():
    if "nc" not in _cache:
        _cache["nc"] = _build()
    return _cache["nc"]


def _shard_inputs(x, w_qkv, w_proj):
    """Build per-core input dicts. Core index = b * 4 + g."""
    in_maps = []
    for b in range(B):
        xTb = np.ascontiguousarray(x[b].T)  # [C, N]
        for g in range(4):
            r = g * 256  # head-group row offset within each of q/k/v sections
            wqkT = np.empty((C, 512), np.float32)
            wqkT[:, 0:128] = w_qkv[r:r + 128].T                  # q pair 0
            wqkT[:, 128:256] = w_qkv[C + r:C + r + 128].T        # k pair 0
            wqkT[:, 256:384] = w_qkv[r + 128:r + 256].T          # q pair 1
            wqkT[:, 384:512] = w_qkv[C + r + 128:C + r + 256].T  # k pair 1
            wvT = np.ascontiguousarray(w_qkv[2 * C + r:2 * C + r + 256].T)
            wpT = np.ascontiguousarray(w_proj[:, r:r + 256].T)
            in_maps.append({
                "xT": xTb,
                "wqkT": wqkT,
                "wvT": wvT,
                "wpT": wpT,
            })
    return in_maps


def kernel(x, w_qkv, w_proj, b_proj, _trace=False):
    from concourse.bass_utils import run_bass_kernel_spmd

    x = np.asarray(x, dtype=np.float32)
    w_qkv = np.asarray(w_qkv, dtype=np.float32)
    w_proj = np.asarray(w_proj, dtype=np.float32)
    b_proj = np.asarray(b_proj, dtype=np.float32)

    nc = _get_nc()
    in_maps = _shard_inputs(x, w_qkv, w_proj)
    res = run_bass_kernel_spmd(
        nc, in_maps, core_ids=list(range(N_CORES)), trace=_trace
    )
    out = np.zeros((B, N, C), np.float32)
    for b in range(B):
        for g in range(4):
            out[b] += res.results[b * 4 + g]["out"]
    out += b_proj
    if _trace:
        _cache["last_exec_time_ns"] = res.exec_time_ns
        _cache["last_results"] = res
    return out
